# revision 22
# baseline (speedup 1.0000x reference)
"""GAT (2-layer, DGL-style) on 8 Trainium2 NeuronCores.

Strategy
--------
- Shard destination nodes (2500/core, padded to 2560 = 20 windows x 128).
  Each core owns all edges pointing at its nodes, sorted by dst.
- Node-level compute (feat = x @ W, attention logits el/er) runs sharded;
  feature tables are AllGathered so every core can gather any src row.
- Edge aggregation per 128-dst window: per-edge rows are fetched with
  dma_gather (feat in bf16, logits in fp32); softmax numerator ex =
  exp(leaky_relu(el_src + er_dst)) is computed without the max-shift
  (mathematically identical, range-safe here); messages ex*feat are
  reduced into PSUM via one-hot selector matmuls S^T @ msg, and the
  1/sum(ex) normalization is applied once per window.
"""

import os
import numpy as np
import ml_dtypes

import concourse.bass as bass
import concourse.bacc as bacc
import concourse.tile as tile
import concourse.mybir as mybir
from concourse.bass_utils import run_bass_kernel_spmd

F32 = mybir.dt.float32
BF16 = mybir.dt.bfloat16
I16 = mybir.dt.int16
AF = mybir.ActivationFunctionType
OP = mybir.AluOpType

# problem constants (fixed by the harness)
N, E, IN_DIM, HID, OUT = 20000, 320000, 256, 128, 64
H0, D0 = 4, 128
F0 = H0 * D0            # 512
NEG = 0.2
NCORES = 8
NS = N // NCORES        # 2500 owned nodes per core
WSZ = 128
NW = 20                 # windows per core (20*128 = 2560 >= 2500)
NSP = NW * WSZ          # 2560 padded local rows
NG = NCORES * NSP       # 20480 global padded rows

# table widths (elements); every gathered row is a multiple of 256 bytes.
# row layout: [feat | el | pad]; er never leaves the owning core.
FE1_W = 640             # bf16 -> 1280 B (feat 0:512, el 512:516)
FE2_W = 128             # bf16 -> 256 B  (feat2 0:64, el2 64:65)


# --------------------------------------------------------------------------
# host-side graph prep (index/layout work only)
# --------------------------------------------------------------------------

def _pack_idx(ids: np.ndarray) -> np.ndarray:
    """Pack an index list for dma_gather: position i -> partition i%16,
    col i//16, replicated across the 8 groups of 16 partitions."""
    n = ids.shape[0]
    assert n % 16 == 0
    t = ids.reshape(n // 16, 16).T.astype(np.int16)     # [16, n//16]
    return np.tile(t, (8, 1))                            # [128, n//16]


def _prep_core(src_c, dst_local, kblk):
    """Per-core edge layout: windows of 128 dsts, KBLK blocks of 128 slots."""
    nslot = NW * kblk * 128
    src_rows = np.zeros(nslot, np.int64)
    dst_rows = np.zeros(nslot, np.int64)
    s_tab = np.zeros((NW, 128, kblk * 128), np.float32)

    w_of_edge = dst_local // WSZ
    for w in range(NW):
        m = w_of_edge == w
        cnt = int(m.sum())
        assert cnt <= kblk * 128
        base = w * kblk * 128
        i = np.arange(cnt)
        sw = src_c[m]
        lw = sw % NS
        src_rows[base + i] = (lw // 128) * (NCORES * 128) + (sw // NS) * 128 + lw % 128
        dst_rows[base + i] = dst_local[m]                   # local id < 2500
        dloc = dst_local[m] - w * WSZ                       # 0..127
        s_tab[w, i % 128, (i // 128) * 128 + dloc] = 1.0

    return src_rows, dst_rows, s_tab


def _host_prep(src, dst):
    src = np.asarray(src).astype(np.int64)
    dst = np.asarray(dst).astype(np.int64)
    order = np.argsort(dst, kind="stable")
    src, dst = src[order], dst[order]

    cores = []
    kblk = 0
    for c in range(NCORES):
        m = (dst // NS) == c
        dl = dst[m] - c * NS
        wcnt = np.bincount(dl // WSZ, minlength=NW)
        kblk = max(kblk, int(np.ceil(wcnt.max() / 128)))
        cores.append((src[m], dl))
    out = []
    for c in range(NCORES):
        s_c, dl = cores[c]
        sr, dr, s_tab = _prep_core(s_c, dl, kblk)
        s4 = s_tab.reshape(NW, 128, kblk, 128)
        st_tab = np.ascontiguousarray(s4.transpose(0, 3, 2, 1)).reshape(
            NW, 128, kblk * 128)
        out.append({
            "isrc": _pack_idx(sr),
            "s": s_tab.astype(ml_dtypes.bfloat16),
            "st": st_tab.astype(ml_dtypes.bfloat16),
        })
    return out, kblk



# --------------------------------------------------------------------------
# Tile assigns DMASW completion sems round-robin over all Pool DMA
# instructions, but the SWDGE ucode locks each sem to the queue that first
# uses it. With multiple SWDGE queues the rotation mixes queues on one sem.
# Patch the lane choice to be queue-keyed: queue q owns lanes {2q, 2q+1}.
import concourse.tile_sem_assignment as _tsa


def _queue_keyed_assign_tick(self, inst):
    eng = inst.engine
    if (isinstance(inst, _tsa.DMAInst)
            and not isinstance(inst, _tsa.bass_isa.UserSyncedRemoteDMADescs)
            and eng == mybir.EngineType.Pool):
        q = int(getattr(inst, "queue_num", 0) or 0)
        cnt = getattr(self, "_per_q_cnt", None)
        if cnt is None:
            cnt = self._per_q_cnt = [0, 0, 0, 0]
        lane = (2 * q + cnt[q] % 2) % 8
        cnt[q] += 1
        self.next_sw_dma_idx = lane
        res = _tsa.TileClockTick._assign_tick_orig(self, inst)
        return res
    return _tsa.TileClockTick._assign_tick_orig(self, inst)


if not hasattr(_tsa.TileClockTick, "_assign_tick_orig"):
    _tsa.TileClockTick._assign_tick_orig = _tsa.TileClockTick._assign_tick
    _tsa.TileClockTick._assign_tick = _queue_keyed_assign_tick


# --------------------------------------------------------------------------
# device program
# --------------------------------------------------------------------------

def _build(kblk: int, level: int = 0):
    # level: 0=full, 1=node compute + allgather only, 2=+layer0 edge phase,
    #        3=+transition (no layer-1 edge phase)
    nc = bacc.Bacc(None, target_bir_lowering=False, num_devices=NCORES,
                   num_swdge_queues=4)
    nbc = kblk * 128                                        # S cols per window

    xT = nc.dram_tensor("xT", [IN_DIM, NSP], F32, kind="ExternalInput")
    w1 = nc.dram_tensor("w1", [IN_DIM, F0], F32, kind="ExternalInput")
    alr1 = nc.dram_tensor("alr1", [F0, 2 * H0], F32, kind="ExternalInput")
    b1r = nc.dram_tensor("b1r", [1, F0], F32, kind="ExternalInput")
    w2 = nc.dram_tensor("w2", [F0, OUT], F32, kind="ExternalInput")
    alr2 = nc.dram_tensor("alr2", [OUT, 2], F32, kind="ExternalInput")
    b2r = nc.dram_tensor("b2r", [1, OUT], F32, kind="ExternalInput")
    stab = nc.dram_tensor("stab", [NW, 128, nbc], BF16, kind="ExternalInput")
    sttab = nc.dram_tensor("sttab", [NW, 128, nbc], BF16, kind="ExternalInput")
    isrc = nc.dram_tensor("isrc", [128, NW * kblk * 8], I16, kind="ExternalInput")
    idf = nc.dram_tensor("idf", [128, 128], F32, kind="ExternalInput")
    out = nc.dram_tensor("out", [NS, OUT], F32, kind="ExternalOutput")

    with tile.TileContext(nc) as tc:
        with (
            tc.tile_pool(name="const", bufs=1) as cp,
            tc.tile_pool(name="persist", bufs=1) as pp,
            tc.tile_pool(name="dram", bufs=1, space="DRAM") as dram,
        ):
            fe1_loc = dram.tile([NSP, FE1_W], BF16)
            fe2_loc = dram.tile([NSP, FE2_W], BF16)
            fe1_g = dram.tile([NG, FE1_W], BF16)
            fe2_g = dram.tile([NG, FE2_W], BF16)
            fe1_gs = [dram.tile([NCORES * 128, FE1_W], BF16,
                                addr_space="Shared", tag=f"fe1gs{m}",
                                name=f"fe1gs{m}") for m in range(NW)]
            fe2_gs = [dram.tile([NCORES * 128, FE2_W], BF16,
                                addr_space="Shared", tag=f"fe2gs{m}",
                                name=f"fe2gs{m}") for m in range(NW)]

            # -------- constants --------
            ident = cp.tile([128, 128], F32)
            nc.sync.dma_start(ident[:], idf[:])
            w1sb = [cp.tile([128, F0], F32, tag=f"w1_{k}", name=f"w1sb{k}") for k in range(2)]
            for k in range(2):
                nc.sync.dma_start(w1sb[k][:], w1[k * 128:(k + 1) * 128, :])
            alr1sb = [cp.tile([128, 2 * H0], F32, tag=f"alr1_{f}", name=f"alr1sb{f}") for f in range(4)]
            for f in range(4):
                nc.sync.dma_start(alr1sb[f][:], alr1[f * 128:(f + 1) * 128, :])
            w2sb = [cp.tile([128, OUT], F32, tag=f"w2_{k}", name=f"w2sb{k}") for k in range(4)]
            for k in range(4):
                nc.sync.dma_start(w2sb[k][:], w2[k * 128:(k + 1) * 128, :])
            alr2sb = cp.tile([64, 2], F32)
            nc.sync.dma_start(alr2sb[:], alr2[:])
            b1one = cp.tile([1, F0], F32)
            nc.sync.dma_start(b1one[:], b1r[:])
            b1bc = cp.tile([128, F0], F32)
            nc.gpsimd.partition_broadcast(b1bc[:], b1one[:])
            b2one = cp.tile([1, OUT], F32)
            nc.sync.dma_start(b2one[:], b2r[:])
            b2bc = cp.tile([128, OUT], F32)
            nc.gpsimd.partition_broadcast(b2bc[:], b2one[:])
            isrc_t = cp.tile([128, NW * kblk * 8], I16)
            nc.sync.dma_start(isrc_t[:], isrc[:])

            identb = cp.tile([128, 128], BF16)
            nc.vector.tensor_copy(identb[:], ident[:])
            # local er values per window (dst side never leaves the core)
            er1_sb = pp.tile([128, NW, H0], BF16, tag="er1_sb")
            er2_sb = pp.tile([128, NW, 1], BF16, tag="er2_sb")

            # h^T chunks, filled during the layer-0 edge phase
            hT = [pp.tile([128, NSP], F32, tag=f"hT_{f}", name=f"hT{f}") for f in range(4)]

            # ============= Phase A: layer-0 node compute =============
            with (
                tc.tile_pool(name="pAF", bufs=1) as pf,
                tc.tile_pool(name="pA", bufs=2, space="PSUM") as pA,
                tc.tile_pool(name="sA", bufs=2) as sA,
            ):
                xtsb = [pf.tile([128, NSP], F32, tag=f"xt_{k}", name=f"xt{k}") for k in range(2)]
                for k in range(2):
                    nc.sync.dma_start(xtsb[k][:], xT[k * 128:(k + 1) * 128, :])
                featT = [pf.tile([128, NSP], F32, tag=f"fT_{f}", name=f"fT{f}") for f in range(4)]
                elr1 = pf.tile([8, NSP], F32, tag="elr1")

                for n in range(5):
                    ncol = slice(n * 512, (n + 1) * 512)
                    for f in range(4):
                        ps = pA.tile([128, 512], F32, tag="ft")
                        for k in range(2):
                            nc.tensor.matmul(
                                ps[:], w1sb[k][:, f * 128:(f + 1) * 128],
                                xtsb[k][:, ncol], start=(k == 0), stop=(k == 1))
                        if f % 2 == 0:
                            nc.vector.tensor_copy(featT[f][:, ncol], ps[:])
                        else:
                            nc.scalar.copy(featT[f][:, ncol], ps[:])
                    pse = pA.tile([8, 512], F32, tag="elr")
                    for f in range(4):
                        nc.tensor.matmul(pse[:], alr1sb[f][:], featT[f][:, ncol],
                                         start=(f == 0), stop=(f == 3))
                    nc.vector.tensor_copy(elr1[:, ncol], pse[:])

                # node-major tables via PE transpose
                for m in range(NW):
                    mc = slice(m * 128, (m + 1) * 128)
                    fem = sA.tile([128, FE1_W], BF16, tag="fem")
                    nc.vector.memset(fem[:, F0 + H0:FE1_W], 0.0)
                    for f in range(4):
                        pt = pA.tile([128, 128], F32, tag="tp")
                        nc.tensor.transpose(pt[:], featT[f][:, mc], ident[:])
                        if f % 2 == 0:
                            nc.vector.tensor_copy(fem[:, f * 128:(f + 1) * 128], pt[:])
                        else:
                            nc.scalar.copy(fem[:, f * 128:(f + 1) * 128], pt[:])
                    pte = pA.tile([128, 8], F32, tag="tp")
                    nc.tensor.transpose(pte[:], elr1[:, mc], ident[0:8, 0:8])
                    nc.vector.tensor_copy(fem[:, F0:F0 + H0], pte[:, 0:H0])
                    nc.vector.tensor_copy(er1_sb[:, m, :], pte[:, H0:2 * H0])
                    nc.sync.dma_start(fe1_loc[mc, :], fem[:])

            grp = [list(range(NCORES))]
            for m in range(NW):
                nc.gpsimd.collective_compute(
                    "AllGather", OP.bypass, grp,
                    ins=[fe1_loc[m * 128:(m + 1) * 128, :].opt()],
                    outs=[fe1_gs[m][:].opt()])
                nc.sync.dma_start(
                    fe1_g[m * NCORES * 128:(m + 1) * NCORES * 128, :],
                    fe1_gs[m][:])

            # ============= Phase B: edge aggregation =============
            dbg = int(os.environ.get("K_DBG", "0"))

            def edge_layer(fe_g, er_sb, fe_w, nhead, dfeat, finalize):
                nhf = nhead * dfeat
                el_off = nhead * dfeat
                with (
                    tc.tile_pool(name="pB", bufs=2, space="PSUM") as pB,
                    tc.tile_pool(name="sB", bufs=3) as sB,
                    tc.tile_pool(name="sB3", bufs=3) as sB3,
                ):
                    # absorb table sems into the POOL engine clock so each
                    # dma_gather carries at most one embedded wait
                    dmyb = sB.tile([1, 16], BF16, tag="dmyb")
                    nc.gpsimd.dma_start(dmyb[:], fe_g[0:1, 0:16])

                    for w in range(NW):
                        feg = sB.tile([128, kblk, fe_w], BF16, tag="feg")
                        qk = [kblk // 4, kblk // 2, 3 * kblk // 4, kblk]
                        b0 = 0
                        for q in range(4):
                            b1 = qk[q]
                            if b1 == b0:
                                continue
                            qcol = slice(w * kblk * 8 + b0 * 8,
                                         w * kblk * 8 + b1 * 8)
                            nc.gpsimd.dma_gather(
                                feg[:, b0:b1, :], fe_g[:], isrc_t[:, qcol],
                                num_idxs=(b1 - b0) * 128,
                                num_idxs_reg=(b1 - b0) * 128,
                                elem_size=fe_w, single_packet=False,
                                queue_num=q)
                            b0 = b1
                        s_t = sB.tile([128, nbc], BF16, tag="s_t")
                        nc.sync.dma_start(s_t[:], stab[w])
                        st_t = sB.tile([128, nbc], BF16, tag="st_t")
                        nc.sync.dma_start(st_t[:], sttab[w])

                        # er expansion: er_e(block b) = S_b @ er_win, with
                        # S^T from the host (dst values stay on-core)
                        er_ps = pB.tile([128, kblk * nhead], F32, tag="er_ps")
                        for b in range(kblk):
                            bs = slice(b * 128, (b + 1) * 128)
                            nc.tensor.matmul(
                                er_ps[:, b * nhead:(b + 1) * nhead],
                                st_t[:, bs], er_sb[:, w, :],
                                start=True, stop=True)

                        epre = sB.tile([128, kblk * nhead], F32, tag="epre")
                        nc.vector.tensor_tensor(
                            out=epre[:],
                            in0=feg[:, :, el_off:el_off + nhead],
                            in1=er_ps[:].rearrange("p (b h) -> p b h", h=nhead),
                            op=OP.add)
                        # leaky_relu = max(x,0) + NEG*min(x,0)  (Lrelu LUT is
                        # not implemented in CoreSim; DVE decomposition is cheap)
                        lpos = sB.tile([128, kblk * nhead], F32, tag="lpos")
                        nc.vector.tensor_scalar(out=lpos[:], in0=epre[:],
                                                scalar1=0.0, scalar2=None,
                                                op0=OP.max)
                        lneg = sB.tile([128, kblk * nhead], F32, tag="lneg")
                        nc.vector.tensor_scalar(out=lneg[:], in0=epre[:],
                                                scalar1=0.0, scalar2=NEG,
                                                op0=OP.min, op1=OP.mult)
                        exw = sB.tile([128, kblk * nhead], F32, tag="exw")
                        nc.vector.tensor_tensor(out=exw[:], in0=lpos[:],
                                                in1=lneg[:], op=OP.add)
                        nc.scalar.activation(exw[:], exw[:], AF.Exp)
                        # 16 cols per block keeps each rhs slice 32B-aligned
                        exb = sB.tile([128, kblk, 16], BF16, tag="exb")
                        nc.vector.tensor_copy(
                            exb[:, :, 0:nhead],
                            exw[:].rearrange("p (b h) -> p b h", h=nhead))

                        rst = pB.tile([128, nhf], F32, tag="rst")
                        s_ps = pB.tile([128, nhead], F32, tag="s_ps")
                        for b in range(kblk):
                            msg = sB3.tile([128, nhf], BF16, tag="msg")
                            for h in range(nhead):
                                fslc = slice(h * dfeat, (h + 1) * dfeat)
                                sc = exw[:, b * nhead + h:b * nhead + h + 1]
                                if nhead > 1 and h >= 2:
                                    nc.scalar.activation(msg[:, fslc],
                                                         feg[:, b, fslc],
                                                         AF.Copy, scale=sc)
                                else:
                                    nc.vector.tensor_scalar(
                                        out=msg[:, fslc], in0=feg[:, b, fslc],
                                        scalar1=sc, scalar2=None, op0=OP.mult)
                            bs = slice(b * 128, (b + 1) * 128)
                            nc.tensor.matmul(rst[:], s_t[:, bs], msg[:],
                                             start=(b == 0), stop=(b == kblk - 1))
                            nc.tensor.matmul(
                                s_ps[:], s_t[:, bs],
                                exb[:, b, 0:nhead],
                                start=(b == 0), stop=(b == kblk - 1),
                                skip_group_check=True)

                        if dbg == 3:
                            dtmp = sB.tile([128, OUT], F32, tag="dtmp")
                            nc.vector.tensor_copy(dtmp[:], rst[:, 0:OUT])
                            lo, hi = w * 128, min((w + 1) * 128, NS)
                            nc.sync.dma_start(out[lo:hi, :], dtmp[0:hi - lo, :])
                            continue
                        ssb = sB.tile([128, nhead], F32, tag="ssb")
                        nc.vector.tensor_scalar(out=ssb[:], in0=s_ps[:],
                                                scalar1=1e-30, scalar2=None,
                                                op0=OP.max)
                        rec = sB.tile([128, nhead], F32, tag="rec")
                        nc.vector.reciprocal(rec[:], ssb[:])
                        if dbg == 4:
                            dtmp = sB.tile([128, OUT], F32, tag="dtmp")
                            nc.vector.tensor_copy(dtmp[:], rst[:, 0:OUT])
                            lo, hi = w * 128, min((w + 1) * 128, NS)
                            nc.sync.dma_start(out[lo:hi, :], dtmp[0:hi - lo, :])
                            continue
                        finalize(w, rst, rec, pB, sB)

            # ---- layer 0 finalize: 1/s, +b1, ELU, transpose into hT ----
            def fin0(w, rst, rec, pB, sB):
                mc = slice(w * 128, (w + 1) * 128)
                hsb = sB.tile([128, F0], F32, tag="hsb")
                for h in range(H0):
                    fslc = slice(h * D0, (h + 1) * D0)
                    sc = rec[:, h:h + 1]
                    if h % 2 == 0:
                        nc.vector.tensor_scalar(out=hsb[:, fslc], in0=rst[:, fslc],
                                                scalar1=sc, scalar2=None,
                                                op0=OP.mult)
                    else:
                        nc.scalar.activation(hsb[:, fslc], rst[:, fslc],
                                             AF.Copy, scale=sc)
                nc.vector.tensor_tensor(out=hsb[:], in0=hsb[:], in1=b1bc[:],
                                        op=OP.add)
                # ELU(x) = (max(x,0)-1) + exp(min(x,0))
                pos = sB.tile([128, F0], F32, tag="pos")
                nc.vector.tensor_scalar(out=pos[:], in0=hsb[:], scalar1=0.0,
                                        scalar2=-1.0, op0=OP.max, op1=OP.add)
                negx = sB.tile([128, F0], F32, tag="negx")
                nc.vector.tensor_scalar(out=negx[:], in0=hsb[:], scalar1=0.0,
                                        scalar2=None, op0=OP.min)
                expn = sB.tile([128, F0], F32, tag="expn")
                nc.scalar.activation(expn[:], negx[:], AF.Exp)
                nc.vector.tensor_tensor(out=hsb[:], in0=pos[:], in1=expn[:],
                                        op=OP.add)
                for f in range(4):
                    pt = pB.tile([128, 128], F32, tag="tp0")
                    nc.tensor.transpose(pt[:], hsb[:, f * 128:(f + 1) * 128],
                                        ident[:])
                    if f % 2 == 0:
                        nc.vector.tensor_copy(hT[f][:, mc], pt[:])
                    else:
                        nc.scalar.copy(hT[f][:, mc], pt[:])

            if level == 1:
                ocp = pp.tile([128, OUT], F32, tag="ocp", name="ocp")
                for m in range(NW):
                    lo, hi = m * 128, min((m + 1) * 128, NS)
                    nc.sync.dma_start(ocp[0:hi - lo, :], ee1_g[lo:hi, :])
                    nc.sync.dma_start(out[lo:hi, :], ocp[0:hi - lo, :])
            if level >= 2 or level == 0:
                edge_layer(fe1_g, er1_sb, FE1_W, H0, D0, fin0)

            # ============= transition: layer-1 node compute =============
            if level == 2 and dbg == 0:
                ocp2 = pp.tile([128, OUT], F32, tag="ocp", name="ocp2")
                for m in range(NW):
                    lo, hi = m * 128, min((m + 1) * 128, NS)
                    nc.vector.tensor_copy(ocp2[0:hi - lo, :],
                                          hT[0][:, lo:hi][0:hi - lo, 0:OUT])
                    nc.sync.dma_start(out[lo:hi, :], ocp2[0:hi - lo, :])

            if level == 0 or level >= 3:
                with (
                    tc.tile_pool(name="pTF", bufs=1) as ptf,
                    tc.tile_pool(name="pT", bufs=2, space="PSUM") as pT,
                    tc.tile_pool(name="sT", bufs=2) as sT,
                ):
                    f2T = ptf.tile([64, NSP], F32, tag="f2T")
                    elr2 = ptf.tile([2, NSP], F32, tag="elr2")
                    for n in range(5):
                        ncol = slice(n * 512, (n + 1) * 512)
                        ps = pT.tile([64, 512], F32, tag="f2")
                        for k in range(4):
                            nc.tensor.matmul(ps[:], w2sb[k][:], hT[k][:, ncol],
                                             start=(k == 0), stop=(k == 3))
                        nc.vector.tensor_copy(f2T[:, ncol], ps[:])
                        pse = pT.tile([2, 512], F32, tag="el2")
                        nc.tensor.matmul(pse[:], alr2sb[:], f2T[:, ncol],
                                         start=True, stop=True)
                        nc.scalar.copy(elr2[:, ncol], pse[:])
                    for m in range(NW):
                        mc = slice(m * 128, (m + 1) * 128)
                        fem = sT.tile([128, FE2_W], BF16, tag="fem2")
                        nc.vector.memset(fem[:, 65:FE2_W], 0.0)
                        pt = pT.tile([128, 64], F32, tag="tpf2")
                        nc.tensor.transpose(pt[:], f2T[:, mc], ident[0:64, 0:64])
                        nc.vector.tensor_copy(fem[:, 0:64], pt[:])
                        pte = pT.tile([128, 2], F32, tag="tpf2")
                        nc.tensor.transpose(pte[:], elr2[:, mc], ident[0:2, 0:2])
                        nc.vector.tensor_copy(fem[:, 64:65], pte[:, 0:1])
                        nc.vector.tensor_copy(er2_sb[:, m, :], pte[:, 1:2])
                        nc.sync.dma_start(fe2_loc[mc, 0:FE2_W], fem[:])

                for m in range(NW):
                    nc.gpsimd.collective_compute(
                        "AllGather", OP.bypass, grp,
                        ins=[fe2_loc[m * 128:(m + 1) * 128, :].opt()],
                        outs=[fe2_gs[m][:].opt()])
                    nc.sync.dma_start(
                        fe2_g[m * NCORES * 128:(m + 1) * NCORES * 128, :],
                        fe2_gs[m][:])

                # ---- layer 1 finalize: 1/s, +b2, write output rows ----
                def fin1(w, rst, rec, pB, sB):
                    osb = sB.tile([128, OUT], F32, tag="osb")
                    nc.vector.tensor_scalar(out=osb[:], in0=rst[:],
                                            scalar1=rec[:, 0:1],
                                            scalar2=None, op0=OP.mult)
                    nc.vector.tensor_tensor(out=osb[:], in0=osb[:], in1=b2bc[:],
                                            op=OP.add)
                    lo, hi = w * 128, min((w + 1) * 128, NS)
                    nc.sync.dma_start(out[lo:hi, :], osb[0:hi - lo, :])

                if level == 0:
                    edge_layer(fe2_g, er2_sb, FE2_W, 1, OUT, fin1)

    nc.compile()
    return nc


# --------------------------------------------------------------------------
# entry point
# --------------------------------------------------------------------------

LAST_RESULTS = None

def kernel(x, src, dst, W1, al1, ar1, b1, W2, al2, ar2, b2):
    x = np.asarray(x, np.float32)
    W1 = np.ascontiguousarray(np.asarray(W1, np.float32))
    W2 = np.ascontiguousarray(np.asarray(W2, np.float32))
    al1 = np.asarray(al1, np.float32)
    ar1 = np.asarray(ar1, np.float32)
    al2 = np.asarray(al2, np.float32)
    ar2 = np.asarray(ar2, np.float32)
    b1 = np.asarray(b1, np.float32)
    b2 = np.asarray(b2, np.float32)

    per_core, kblk = _host_prep(src, dst)
    nc = _build(kblk, level=int(os.environ.get('K_LEVEL', '0')))

    # weight layouts (pure placement, no arithmetic)
    alr1 = np.zeros((F0, 2 * H0), np.float32)
    for h in range(H0):
        alr1[h * D0:(h + 1) * D0, h] = al1[h]
        alr1[h * D0:(h + 1) * D0, H0 + h] = ar1[h]
    alr2 = np.zeros((OUT, 2), np.float32)
    alr2[:, 0] = al2[0]
    alr2[:, 1] = ar2[0]
    ident = np.eye(128, dtype=np.float32)

    in_maps = []
    for c in range(NCORES):
        xc = np.zeros((IN_DIM, NSP), np.float32)
        xc[:, :NS] = x[c * NS:(c + 1) * NS].T
        in_maps.append({
            "xT": np.ascontiguousarray(xc),
            "w1": W1, "alr1": alr1, "b1r": b1.reshape(1, F0).copy(),
            "w2": W2, "alr2": alr2, "b2r": b2.reshape(1, OUT).copy(),
            "stab": per_core[c]["s"],
            "sttab": per_core[c]["st"],
            "isrc": per_core[c]["isrc"],
            "idf": ident,
        })

    trace = bool(int(os.environ.get("K_TRACE", "0")))
    res = run_bass_kernel_spmd(nc, in_maps, core_ids=list(range(NCORES)),
                               trace=trace)
    global LAST_RESULTS
    LAST_RESULTS = res
    return np.concatenate([res.results[c]["out"] for c in range(NCORES)], axis=0)


# revision 26
# speedup vs baseline: 1.3224x; 1.3224x over previous
"""GAT (2-layer, DGL-style) on 8 Trainium2 NeuronCores.

Strategy
--------
- Shard destination nodes (2500/core, padded to 2560 = 20 windows x 128).
  Each core owns all edges pointing at its nodes, sorted by dst.
- Node-level compute (feat = x @ W, attention logits el/er) runs sharded;
  feature tables are AllGathered so every core can gather any src row.
- Edge aggregation per 128-dst window: per-edge rows are fetched with
  dma_gather (feat in bf16, logits in fp32); softmax numerator ex =
  exp(leaky_relu(el_src + er_dst)) is computed without the max-shift
  (mathematically identical, range-safe here); messages ex*feat are
  reduced into PSUM via one-hot selector matmuls S^T @ msg, and the
  1/sum(ex) normalization is applied once per window.
"""

import os
import numpy as np
import ml_dtypes

import concourse.bass as bass
import concourse.bacc as bacc
import concourse.tile as tile
import concourse.mybir as mybir
from concourse.bass_utils import run_bass_kernel_spmd

F32 = mybir.dt.float32
BF16 = mybir.dt.bfloat16
I16 = mybir.dt.int16
AF = mybir.ActivationFunctionType
OP = mybir.AluOpType

# problem constants (fixed by the harness)
N, E, IN_DIM, HID, OUT = 20000, 320000, 256, 128, 64
H0, D0 = 4, 128
F0 = H0 * D0            # 512
NEG = 0.2
NCORES = 8
NS = N // NCORES        # 2500 owned nodes per core
WSZ = 128
NW = 20                 # windows per core (20*128 = 2560 >= 2500)
NSP = NW * WSZ          # 2560 padded local rows
NG = NCORES * NSP       # 20480 global padded rows

# table widths (elements); every gathered row is a multiple of 256 bytes.
# row layout: [feat | el | pad]; er never leaves the owning core.
FE1_W = 640             # bf16 -> 1280 B (feat 0:512, el 512:516)
FE2_W = 128             # bf16 -> 256 B  (feat2 0:64, el2 64:65)


# --------------------------------------------------------------------------
# host-side graph prep (index/layout work only)
# --------------------------------------------------------------------------

def _pack_idx(ids: np.ndarray) -> np.ndarray:
    """Pack an index list for dma_gather: position i -> partition i%16,
    col i//16, replicated across the 8 groups of 16 partitions."""
    n = ids.shape[0]
    assert n % 16 == 0
    t = ids.reshape(n // 16, 16).T.astype(np.int16)     # [16, n//16]
    return np.tile(t, (8, 1))                            # [128, n//16]


def _prep_core(src_c, dst_local, kblk):
    """Per-core edge layout: windows of 128 dsts, KBLK blocks of 128 slots."""
    nslot = NW * kblk * 128
    src_rows = np.zeros(nslot, np.int64)
    dst_rows = np.zeros(nslot, np.int64)
    s_tab = np.zeros((NW, 128, kblk * 128), np.float32)

    w_of_edge = dst_local // WSZ
    for w in range(NW):
        m = w_of_edge == w
        cnt = int(m.sum())
        assert cnt <= kblk * 128
        base = w * kblk * 128
        i = np.arange(cnt)
        sw = src_c[m]
        src_rows[base + i] = (sw // NS) * NSP + (sw % NS)   # global padded row
        dst_rows[base + i] = dst_local[m]                   # local id < 2500
        dloc = dst_local[m] - w * WSZ                       # 0..127
        s_tab[w, i % 128, (i // 128) * 128 + dloc] = 1.0

    return src_rows, dst_rows, s_tab


def _host_prep(src, dst):
    src = np.asarray(src).astype(np.int64)
    dst = np.asarray(dst).astype(np.int64)
    order = np.argsort(dst, kind="stable")
    src, dst = src[order], dst[order]

    cores = []
    kblk = 0
    for c in range(NCORES):
        m = (dst // NS) == c
        dl = dst[m] - c * NS
        wcnt = np.bincount(dl // WSZ, minlength=NW)
        kblk = max(kblk, int(np.ceil(wcnt.max() / 128)))
        cores.append((src[m], dl))
    out = []
    for c in range(NCORES):
        s_c, dl = cores[c]
        sr, dr, s_tab = _prep_core(s_c, dl, kblk)
        s4 = s_tab.reshape(NW, 128, kblk, 128)
        st_tab = np.ascontiguousarray(s4.transpose(0, 3, 2, 1)).reshape(
            NW, 128, kblk * 128)
        out.append({
            "isrc": _pack_idx(sr),
            "s": s_tab.astype(ml_dtypes.bfloat16),
            "st": st_tab.astype(ml_dtypes.bfloat16),
        })
    return out, kblk



# --------------------------------------------------------------------------
# Tile assigns DMASW completion sems round-robin over all Pool DMA
# instructions, but the SWDGE ucode locks each sem to the queue that first
# uses it. With multiple SWDGE queues the rotation mixes queues on one sem.
# Patch the lane choice to be queue-keyed: queue q owns lanes {2q, 2q+1}.
import concourse.tile_sem_assignment as _tsa


def _queue_keyed_assign_tick(self, inst):
    eng = inst.engine
    if (isinstance(inst, _tsa.DMAInst)
            and not isinstance(inst, _tsa.bass_isa.UserSyncedRemoteDMADescs)
            and eng == mybir.EngineType.Pool):
        q = int(getattr(inst, "queue_num", 0) or 0)
        cnt = getattr(self, "_per_q_cnt", None)
        if cnt is None:
            cnt = self._per_q_cnt = [0, 0, 0, 0]
        lane = (2 * q + cnt[q] % 2) % 8
        cnt[q] += 1
        self.next_sw_dma_idx = lane
        res = _tsa.TileClockTick._assign_tick_orig(self, inst)
        return res
    return _tsa.TileClockTick._assign_tick_orig(self, inst)


if not hasattr(_tsa.TileClockTick, "_assign_tick_orig"):
    _tsa.TileClockTick._assign_tick_orig = _tsa.TileClockTick._assign_tick
    _tsa.TileClockTick._assign_tick = _queue_keyed_assign_tick


# --------------------------------------------------------------------------
# device program
# --------------------------------------------------------------------------

def _build(kblk: int, level: int = 0):
    # level: 0=full, 1=node compute + allgather only, 2=+layer0 edge phase,
    #        3=+transition (no layer-1 edge phase)
    nc = bacc.Bacc(None, target_bir_lowering=False, num_devices=NCORES,
                   num_swdge_queues=4)
    nbc = kblk * 128                                        # S cols per window

    xT = nc.dram_tensor("xT", [IN_DIM, NSP], F32, kind="ExternalInput")
    w1 = nc.dram_tensor("w1", [IN_DIM, F0], F32, kind="ExternalInput")
    alr1 = nc.dram_tensor("alr1", [F0, 2 * H0], F32, kind="ExternalInput")
    b1r = nc.dram_tensor("b1r", [1, F0], F32, kind="ExternalInput")
    w2 = nc.dram_tensor("w2", [F0, OUT], F32, kind="ExternalInput")
    alr2 = nc.dram_tensor("alr2", [OUT, 2], F32, kind="ExternalInput")
    b2r = nc.dram_tensor("b2r", [1, OUT], F32, kind="ExternalInput")
    stab = nc.dram_tensor("stab", [NW, 128, nbc], BF16, kind="ExternalInput")
    sttab = nc.dram_tensor("sttab", [NW, 128, nbc], BF16, kind="ExternalInput")
    isrc = nc.dram_tensor("isrc", [128, NW * kblk * 8], I16, kind="ExternalInput")
    idf = nc.dram_tensor("idf", [128, 128], F32, kind="ExternalInput")
    out = nc.dram_tensor("out", [NS, OUT], F32, kind="ExternalOutput")

    with tile.TileContext(nc) as tc:
        with (
            tc.tile_pool(name="const", bufs=1) as cp,
            tc.tile_pool(name="persist", bufs=1) as pp,
            tc.tile_pool(name="dram", bufs=1, space="DRAM") as dram,
        ):
            fe1_loc = dram.tile([NSP, FE1_W], BF16)
            fe2_loc = dram.tile([NSP, FE2_W], BF16)
            fe1_g = dram.tile([NG, FE1_W], BF16, addr_space="Shared")
            fe2_g = dram.tile([NG, FE2_W], BF16, addr_space="Shared")

            # -------- constants --------
            ident = cp.tile([128, 128], F32)
            nc.sync.dma_start(ident[:], idf[:])
            w1sb = [cp.tile([128, F0], F32, tag=f"w1_{k}", name=f"w1sb{k}") for k in range(2)]
            for k in range(2):
                nc.sync.dma_start(w1sb[k][:], w1[k * 128:(k + 1) * 128, :])
            alr1sb = [cp.tile([128, 2 * H0], F32, tag=f"alr1_{f}", name=f"alr1sb{f}") for f in range(4)]
            for f in range(4):
                nc.sync.dma_start(alr1sb[f][:], alr1[f * 128:(f + 1) * 128, :])
            w2sb = [cp.tile([128, OUT], F32, tag=f"w2_{k}", name=f"w2sb{k}") for k in range(4)]
            for k in range(4):
                nc.sync.dma_start(w2sb[k][:], w2[k * 128:(k + 1) * 128, :])
            alr2sb = cp.tile([64, 2], F32)
            nc.sync.dma_start(alr2sb[:], alr2[:])
            b1one = cp.tile([1, F0], F32)
            nc.sync.dma_start(b1one[:], b1r[:])
            b1bc = cp.tile([128, F0], F32)
            nc.gpsimd.partition_broadcast(b1bc[:], b1one[:])
            b2one = cp.tile([1, OUT], F32)
            nc.sync.dma_start(b2one[:], b2r[:])
            b2bc = cp.tile([128, OUT], F32)
            nc.gpsimd.partition_broadcast(b2bc[:], b2one[:])
            isrc_t = cp.tile([128, NW * kblk * 8], I16)
            nc.sync.dma_start(isrc_t[:], isrc[:])

            identb = cp.tile([128, 128], BF16)
            nc.vector.tensor_copy(identb[:], ident[:])
            # local er values per window (dst side never leaves the core)
            er1_sb = pp.tile([128, NW, H0], BF16, tag="er1_sb")
            er2_sb = pp.tile([128, NW, 1], BF16, tag="er2_sb")

            # h^T chunks, filled during the layer-0 edge phase
            hT = [pp.tile([128, NSP], F32, tag=f"hT_{f}", name=f"hT{f}") for f in range(4)]

            # ============= Phase A: layer-0 node compute =============
            with (
                tc.tile_pool(name="pAF", bufs=1) as pf,
                tc.tile_pool(name="pA", bufs=2, space="PSUM") as pA,
                tc.tile_pool(name="sA", bufs=2) as sA,
            ):
                xtsb = [pf.tile([128, NSP], F32, tag=f"xt_{k}", name=f"xt{k}") for k in range(2)]
                for k in range(2):
                    nc.sync.dma_start(xtsb[k][:], xT[k * 128:(k + 1) * 128, :])
                featT = [pf.tile([128, NSP], F32, tag=f"fT_{f}", name=f"fT{f}") for f in range(4)]
                elr1 = pf.tile([8, NSP], F32, tag="elr1")

                for n in range(5):
                    ncol = slice(n * 512, (n + 1) * 512)
                    for f in range(4):
                        ps = pA.tile([128, 512], F32, tag="ft")
                        for k in range(2):
                            nc.tensor.matmul(
                                ps[:], w1sb[k][:, f * 128:(f + 1) * 128],
                                xtsb[k][:, ncol], start=(k == 0), stop=(k == 1))
                        if f % 2 == 0:
                            nc.vector.tensor_copy(featT[f][:, ncol], ps[:])
                        else:
                            nc.scalar.copy(featT[f][:, ncol], ps[:])
                    pse = pA.tile([8, 512], F32, tag="elr")
                    for f in range(4):
                        nc.tensor.matmul(pse[:], alr1sb[f][:], featT[f][:, ncol],
                                         start=(f == 0), stop=(f == 3))
                    nc.vector.tensor_copy(elr1[:, ncol], pse[:])

                # node-major tables via PE transpose
                for m in range(NW):
                    mc = slice(m * 128, (m + 1) * 128)
                    fem = sA.tile([128, FE1_W], BF16, tag="fem")
                    nc.vector.memset(fem[:, F0 + H0:FE1_W], 0.0)
                    for f in range(4):
                        pt = pA.tile([128, 128], F32, tag="tp")
                        nc.tensor.transpose(pt[:], featT[f][:, mc], ident[:])
                        if f % 2 == 0:
                            nc.vector.tensor_copy(fem[:, f * 128:(f + 1) * 128], pt[:])
                        else:
                            nc.scalar.copy(fem[:, f * 128:(f + 1) * 128], pt[:])
                    pte = pA.tile([128, 8], F32, tag="tp")
                    nc.tensor.transpose(pte[:], elr1[:, mc], ident[0:8, 0:8])
                    nc.vector.tensor_copy(fem[:, F0:F0 + H0], pte[:, 0:H0])
                    nc.vector.tensor_copy(er1_sb[:, m, :], pte[:, H0:2 * H0])
                    nc.sync.dma_start(fe1_loc[mc, :], fem[:])

            grp = [list(range(NCORES))]
            nc.gpsimd.collective_compute("AllGather", OP.bypass, grp,
                                         ins=[fe1_loc[:].opt()],
                                         outs=[fe1_g[:].opt()])

            # ============= Phase B: edge aggregation =============
            dbg = int(os.environ.get("K_DBG", "0"))

            def edge_layer(fe_g, er_sb, fe_w, nhead, dfeat, finalize):
                nhf = nhead * dfeat
                el_off = nhead * dfeat
                with (
                    tc.tile_pool(name="pB", bufs=2, space="PSUM") as pB,
                    tc.tile_pool(name="sB", bufs=3) as sB,
                    tc.tile_pool(name="sB3", bufs=3) as sB3,
                ):
                    # absorb table sems into the POOL engine clock so each
                    # dma_gather carries at most one embedded wait
                    dmyb = sB.tile([1, 16], BF16, tag="dmyb")
                    nc.gpsimd.dma_start(dmyb[:], fe_g[0:1, 0:16])

                    for w in range(NW):
                        feg = sB.tile([128, kblk, fe_w], BF16, tag="feg")
                        qk = [kblk // 4, kblk // 2, 3 * kblk // 4, kblk]
                        b0 = 0
                        for q in range(4):
                            b1 = qk[q]
                            if b1 == b0:
                                continue
                            qcol = slice(w * kblk * 8 + b0 * 8,
                                         w * kblk * 8 + b1 * 8)
                            nc.gpsimd.dma_gather(
                                feg[:, b0:b1, :], fe_g[:], isrc_t[:, qcol],
                                num_idxs=(b1 - b0) * 128,
                                num_idxs_reg=(b1 - b0) * 128,
                                elem_size=fe_w, single_packet=False,
                                queue_num=q)
                            b0 = b1
                        s_t = sB.tile([128, nbc], BF16, tag="s_t")
                        nc.sync.dma_start(s_t[:], stab[w])
                        st_t = sB.tile([128, nbc], BF16, tag="st_t")
                        nc.sync.dma_start(st_t[:], sttab[w])

                        # er expansion: er_e(block b) = S_b @ er_win, with
                        # S^T from the host (dst values stay on-core)
                        er_ps = pB.tile([128, kblk * nhead], F32, tag="er_ps")
                        for b in range(kblk):
                            bs = slice(b * 128, (b + 1) * 128)
                            nc.tensor.matmul(
                                er_ps[:, b * nhead:(b + 1) * nhead],
                                st_t[:, bs], er_sb[:, w, :],
                                start=True, stop=True)

                        epre = sB.tile([128, kblk * nhead], F32, tag="epre")
                        nc.vector.tensor_tensor(
                            out=epre[:],
                            in0=feg[:, :, el_off:el_off + nhead],
                            in1=er_ps[:].rearrange("p (b h) -> p b h", h=nhead),
                            op=OP.add)
                        # leaky_relu = max(x,0) + NEG*min(x,0)  (Lrelu LUT is
                        # not implemented in CoreSim; DVE decomposition is cheap)
                        lpos = sB.tile([128, kblk * nhead], F32, tag="lpos")
                        nc.vector.tensor_scalar(out=lpos[:], in0=epre[:],
                                                scalar1=0.0, scalar2=None,
                                                op0=OP.max)
                        lneg = sB.tile([128, kblk * nhead], F32, tag="lneg")
                        nc.vector.tensor_scalar(out=lneg[:], in0=epre[:],
                                                scalar1=0.0, scalar2=NEG,
                                                op0=OP.min, op1=OP.mult)
                        exw = sB.tile([128, kblk * nhead], F32, tag="exw")
                        nc.vector.tensor_tensor(out=exw[:], in0=lpos[:],
                                                in1=lneg[:], op=OP.add)
                        nc.scalar.activation(exw[:], exw[:], AF.Exp)
                        # 16 cols per block keeps each rhs slice 32B-aligned
                        exb = sB.tile([128, kblk, 16], BF16, tag="exb")
                        nc.vector.tensor_copy(
                            exb[:, :, 0:nhead],
                            exw[:].rearrange("p (b h) -> p b h", h=nhead))

                        rst = pB.tile([128, nhf], F32, tag="rst")
                        s_ps = pB.tile([128, nhead], F32, tag="s_ps")
                        for b in range(kblk):
                            msg = sB3.tile([128, nhf], BF16, tag="msg")
                            for h in range(nhead):
                                fslc = slice(h * dfeat, (h + 1) * dfeat)
                                sc = exw[:, b * nhead + h:b * nhead + h + 1]
                                nc.vector.tensor_scalar(
                                    out=msg[:, fslc], in0=feg[:, b, fslc],
                                    scalar1=sc, scalar2=None, op0=OP.mult)
                            bs = slice(b * 128, (b + 1) * 128)
                            nc.tensor.matmul(rst[:], s_t[:, bs], msg[:],
                                             start=(b == 0), stop=(b == kblk - 1))
                            nc.tensor.matmul(
                                s_ps[:], s_t[:, bs],
                                exb[:, b, 0:nhead],
                                start=(b == 0), stop=(b == kblk - 1),
                                skip_group_check=True)

                        if dbg == 3:
                            dtmp = sB.tile([128, OUT], F32, tag="dtmp")
                            nc.vector.tensor_copy(dtmp[:], rst[:, 0:OUT])
                            lo, hi = w * 128, min((w + 1) * 128, NS)
                            nc.sync.dma_start(out[lo:hi, :], dtmp[0:hi - lo, :])
                            continue
                        ssb = sB.tile([128, nhead], F32, tag="ssb")
                        nc.vector.tensor_scalar(out=ssb[:], in0=s_ps[:],
                                                scalar1=1e-30, scalar2=None,
                                                op0=OP.max)
                        rec = sB.tile([128, nhead], F32, tag="rec")
                        nc.vector.reciprocal(rec[:], ssb[:])
                        if dbg == 4:
                            dtmp = sB.tile([128, OUT], F32, tag="dtmp")
                            nc.vector.tensor_copy(dtmp[:], rst[:, 0:OUT])
                            lo, hi = w * 128, min((w + 1) * 128, NS)
                            nc.sync.dma_start(out[lo:hi, :], dtmp[0:hi - lo, :])
                            continue
                        finalize(w, rst, rec, pB, sB)

            # ---- layer 0 finalize: 1/s, +b1, ELU, transpose into hT ----
            def fin0(w, rst, rec, pB, sB):
                mc = slice(w * 128, (w + 1) * 128)
                hsb = sB.tile([128, F0], F32, tag="hsb")
                for h in range(H0):
                    fslc = slice(h * D0, (h + 1) * D0)
                    sc = rec[:, h:h + 1]
                    if h % 2 == 0:
                        nc.vector.tensor_scalar(out=hsb[:, fslc], in0=rst[:, fslc],
                                                scalar1=sc, scalar2=None,
                                                op0=OP.mult)
                    else:
                        nc.scalar.activation(hsb[:, fslc], rst[:, fslc],
                                             AF.Copy, scale=sc)
                nc.vector.tensor_tensor(out=hsb[:], in0=hsb[:], in1=b1bc[:],
                                        op=OP.add)
                # ELU(x) = (max(x,0)-1) + exp(min(x,0))
                pos = sB.tile([128, F0], F32, tag="pos")
                nc.vector.tensor_scalar(out=pos[:], in0=hsb[:], scalar1=0.0,
                                        scalar2=-1.0, op0=OP.max, op1=OP.add)
                negx = sB.tile([128, F0], F32, tag="negx")
                nc.vector.tensor_scalar(out=negx[:], in0=hsb[:], scalar1=0.0,
                                        scalar2=None, op0=OP.min)
                expn = sB.tile([128, F0], F32, tag="expn")
                nc.scalar.activation(expn[:], negx[:], AF.Exp)
                nc.vector.tensor_tensor(out=hsb[:], in0=pos[:], in1=expn[:],
                                        op=OP.add)
                for f in range(4):
                    pt = pB.tile([128, 128], F32, tag="tp0")
                    nc.tensor.transpose(pt[:], hsb[:, f * 128:(f + 1) * 128],
                                        ident[:])
                    if f % 2 == 0:
                        nc.vector.tensor_copy(hT[f][:, mc], pt[:])
                    else:
                        nc.scalar.copy(hT[f][:, mc], pt[:])

            if level == 1:
                ocp = pp.tile([128, OUT], F32, tag="ocp", name="ocp")
                for m in range(NW):
                    lo, hi = m * 128, min((m + 1) * 128, NS)
                    nc.sync.dma_start(ocp[0:hi - lo, :], ee1_g[lo:hi, :])
                    nc.sync.dma_start(out[lo:hi, :], ocp[0:hi - lo, :])
            if level >= 2 or level == 0:
                edge_layer(fe1_g, er1_sb, FE1_W, H0, D0, fin0)

            # ============= transition: layer-1 node compute =============
            if level == 2 and dbg == 0:
                ocp2 = pp.tile([128, OUT], F32, tag="ocp", name="ocp2")
                for m in range(NW):
                    lo, hi = m * 128, min((m + 1) * 128, NS)
                    nc.vector.tensor_copy(ocp2[0:hi - lo, :],
                                          hT[0][:, lo:hi][0:hi - lo, 0:OUT])
                    nc.sync.dma_start(out[lo:hi, :], ocp2[0:hi - lo, :])

            if level == 0 or level >= 3:
                with (
                    tc.tile_pool(name="pTF", bufs=1) as ptf,
                    tc.tile_pool(name="pT", bufs=2, space="PSUM") as pT,
                    tc.tile_pool(name="sT", bufs=2) as sT,
                ):
                    f2T = ptf.tile([64, NSP], F32, tag="f2T")
                    elr2 = ptf.tile([2, NSP], F32, tag="elr2")
                    for n in range(5):
                        ncol = slice(n * 512, (n + 1) * 512)
                        ps = pT.tile([64, 512], F32, tag="f2")
                        for k in range(4):
                            nc.tensor.matmul(ps[:], w2sb[k][:], hT[k][:, ncol],
                                             start=(k == 0), stop=(k == 3))
                        nc.vector.tensor_copy(f2T[:, ncol], ps[:])
                        pse = pT.tile([2, 512], F32, tag="el2")
                        nc.tensor.matmul(pse[:], alr2sb[:], f2T[:, ncol],
                                         start=True, stop=True)
                        nc.scalar.copy(elr2[:, ncol], pse[:])
                    for m in range(NW):
                        mc = slice(m * 128, (m + 1) * 128)
                        fem = sT.tile([128, FE2_W], BF16, tag="fem2")
                        nc.vector.memset(fem[:, 65:FE2_W], 0.0)
                        pt = pT.tile([128, 64], F32, tag="tpf2")
                        nc.tensor.transpose(pt[:], f2T[:, mc], ident[0:64, 0:64])
                        nc.vector.tensor_copy(fem[:, 0:64], pt[:])
                        pte = pT.tile([128, 2], F32, tag="tpf2")
                        nc.tensor.transpose(pte[:], elr2[:, mc], ident[0:2, 0:2])
                        nc.vector.tensor_copy(fem[:, 64:65], pte[:, 0:1])
                        nc.vector.tensor_copy(er2_sb[:, m, :], pte[:, 1:2])
                        nc.sync.dma_start(fe2_loc[mc, 0:FE2_W], fem[:])

                nc.gpsimd.collective_compute(
                    "AllGather", OP.bypass, grp,
                    ins=[fe2_loc[:].opt()], outs=[fe2_g[:].opt()])

                # ---- layer 1 finalize: 1/s, +b2, write output rows ----
                def fin1(w, rst, rec, pB, sB):
                    osb = sB.tile([128, OUT], F32, tag="osb")
                    nc.vector.tensor_scalar(out=osb[:], in0=rst[:],
                                            scalar1=rec[:, 0:1],
                                            scalar2=None, op0=OP.mult)
                    nc.vector.tensor_tensor(out=osb[:], in0=osb[:], in1=b2bc[:],
                                            op=OP.add)
                    lo, hi = w * 128, min((w + 1) * 128, NS)
                    nc.sync.dma_start(out[lo:hi, :], osb[0:hi - lo, :])

                if level == 0:
                    edge_layer(fe2_g, er2_sb, FE2_W, 1, OUT, fin1)

    nc.compile()
    return nc


# --------------------------------------------------------------------------
# entry point
# --------------------------------------------------------------------------

LAST_RESULTS = None

def kernel(x, src, dst, W1, al1, ar1, b1, W2, al2, ar2, b2):
    x = np.asarray(x, np.float32)
    W1 = np.ascontiguousarray(np.asarray(W1, np.float32))
    W2 = np.ascontiguousarray(np.asarray(W2, np.float32))
    al1 = np.asarray(al1, np.float32)
    ar1 = np.asarray(ar1, np.float32)
    al2 = np.asarray(al2, np.float32)
    ar2 = np.asarray(ar2, np.float32)
    b1 = np.asarray(b1, np.float32)
    b2 = np.asarray(b2, np.float32)

    per_core, kblk = _host_prep(src, dst)
    nc = _build(kblk, level=int(os.environ.get('K_LEVEL', '0')))

    # weight layouts (pure placement, no arithmetic)
    alr1 = np.zeros((F0, 2 * H0), np.float32)
    for h in range(H0):
        alr1[h * D0:(h + 1) * D0, h] = al1[h]
        alr1[h * D0:(h + 1) * D0, H0 + h] = ar1[h]
    alr2 = np.zeros((OUT, 2), np.float32)
    alr2[:, 0] = al2[0]
    alr2[:, 1] = ar2[0]
    ident = np.eye(128, dtype=np.float32)

    in_maps = []
    for c in range(NCORES):
        xc = np.zeros((IN_DIM, NSP), np.float32)
        xc[:, :NS] = x[c * NS:(c + 1) * NS].T
        in_maps.append({
            "xT": np.ascontiguousarray(xc),
            "w1": W1, "alr1": alr1, "b1r": b1.reshape(1, F0).copy(),
            "w2": W2, "alr2": alr2, "b2r": b2.reshape(1, OUT).copy(),
            "stab": per_core[c]["s"],
            "sttab": per_core[c]["st"],
            "isrc": per_core[c]["isrc"],
            "idf": ident,
        })

    trace = bool(int(os.environ.get("K_TRACE", "0")))
    res = run_bass_kernel_spmd(nc, in_maps, core_ids=list(range(NCORES)),
                               trace=trace)
    global LAST_RESULTS
    LAST_RESULTS = res
    return np.concatenate([res.results[c]["out"] for c in range(NCORES)], axis=0)
